# revision 69
# baseline (speedup 1.0000x reference)
"""Trainium2 Bass kernel for nn_AudioDeviceModel (18-layer dilated causal CNN).

Data parallel over batch (64 = 8 cores x 8).  Per core, (batch, chan) packs
the 128 SBUF partitions and time is the free dim; each conv tap is one
block-diagonal [128,128]x[128,w] TensorEngine matmul with dilation shifts as
free-dim offsets.

The 1x1 "io" mix and the halved channel-sum skip are folded away algebraically:
  sig_{i+1} = io_i(h_i) + io_b_i + S_i,   S_i = sum_c(sig_i)/2
so conv_{i+1} applied to sig_{i+1} becomes composed taps (io_i @ W_k) read
directly from h_i, plus a 1-channel S path with
  S_{i+1} = u_i . h_i + 8 S_i + c_i,  u_i = io_w_i.sum(1)/2.
Constants migrate into conv biases via gamma_{i+1} = 8 gamma_i + c_i.

The ctrl+S contribution per tap is a 32-row contraction (8 batches x (3 ctrl
+ 1 S)).  Instead of im2col-ing 9 per-dilation blocks from HBM (5.9MB), each
layer contracts ONE [96,W] buffer with a flat K=96 matmul: strip r (rows
32r:32r+32) holds the tap-r data pre-shifted by (2-r)*d for that layer's
dilation.  Three buffers rotate (layer i uses B_{i%3}); strip 2 is always
shift-0 canonical.  Per layer, two small SBUF->SBUF copies re-shift the
static ctrl rows (from the canonical strip 2) and two more shift-replicate
the S row written by the epilogue's recursion (STT writes S~_{i+1} into
B_{(i+2)%3} strip 2).  Layer 0 is the same structure (sig/2 lives in the S
rows; tap weights carry the 2x), so no separate x0 im2col path exists.
(Row-tiled tile_position matmuls would do this without any copies, but any
two matmuls at different row tile positions hang this runtime - probed.)

Epilogue matmuls (S-update at PSUM col-group 32, mixer at col-group 0, which
run concurrently via column tiling) lag the conv stream by two tiles and pop
in two-tile batches so ReLU eviction latency stays off the PE critical path
and adjacent S/mix pairs share the PE-stream transition cost; layer 17's mix
tiles stream the final output eviction chunk-by-chunk.  Layer-0 ReLU
evictions alternate engines, and each layer's last tile splits its eviction
across scalar+vector so the next layer's PSUM reuse does not stall.  Tiles
stride from each layer's receptive-field trim lo (not 512-aligned), which
removes the 2-4-column leading tiles at trimmed layer heads.  Front DMAs
carry only layer 0/1's data; bulk weight/c2 loads are gated behind fake
writes that depend on layer-0 ReLU output, because the SDMA engines
fair-share bandwidth across all outstanding transfers.

Measured: 175.8-177.7us HW exec at the 2.4GHz power profile (baseline
184.8us, original unoptimized 265.8us), rel err 8.0e-3.  The same binary
measures ~210us when the part sits in its ~2.0GHz power state (baseline
220.6us there) - the ratio to baseline is what this kernel controls.
"""

import numpy as np
import ml_dtypes

import concourse.bass as bass
import concourse.tile as tile
from concourse import bacc, mybir
from concourse.bass_utils import run_bass_kernel_spmd

# Problem constants (hardcoded; kernel.py must be self-contained).
DILATIONS = [1, 2, 4, 8, 16, 32, 64, 128, 256, 1, 2, 4, 8, 16, 32, 64, 128, 256]
KSIZE = 3
CH = 16
NUM_SIG = 1
NUM_CTRL = 3
FRAME = 2048
T = 4092
B = 64
NCORES = 8
BL = B // NCORES          # 8 batches per core
W = T
NL = len(DILATIONS)       # 18
TT = 512                  # time tile
NTILES = (T + TT - 1) // TT   # 8 (last tile 508 wide)
MIX_T0 = T - FRAME        # 2044: first time index contributing to output
LAG = 2                   # epilogue lag in tiles

# Receptive-field trim: layer i's output h_i only influences the final frame
# for t >= LO[i]; LO[i] = max(0, LO[i+1] - 2*d_{i+1}) with LO[17] = MIX_T0.
_sums = [0] * NL
_acc = 0
for _i in range(NL - 1, -1, -1):
    _sums[_i] = _acc
    _acc += DILATIONS[_i]
LO = [max(0, MIX_T0 - 2 * _sums[_i]) for _i in range(NL)]

BF16 = ml_dtypes.bfloat16

# Weight bank column layout (bf16, [128, NW]):
#   0:128     layer-0 ctrlS strip block (strip r at rows 32r:32r+32)
#   128:136   u_0
#   136:144   mix_0
#   per i=1..17: taps(384) | ctrlS(128, 3 stacked 32-row strip blocks)
#                | u_i(8, i<=15) | mix_i(8)
_O = {}
_col = 0
_O[("cs", 0)] = 0
_O[("u", 0)] = 128
_O[("mix", 0)] = 136
_col = 144
for _i in range(1, NL):
    _O[("tap", _i)] = _col
    _col += 384
    _O[("cs", _i)] = _col
    _col += 128
    if _i <= 15:
        _O[("u", _i)] = _col
        _col += 8
    _O[("mix", _i)] = _col
    _col += 8
NW = _col                              # 9104
NBIAS = NL + 1                         # 18 conv biases | mixer_b

# CoreSim rejects matmuls whose PSUM window crosses a bank boundary; HW
# handles it fine (the baseline ran bank-crossing mix matmuls for months).
# sim_check.py flips this on to validate the graph in simulation.
SIM_SPLIT_MIX = False
# CoreSim's PSUM pending-zero tracking also dislikes the unaligned (stride-
# from-lo) tile windows; sim_check aligns tiles to 512 for validation runs.
SIM_ALIGNED_TILES = False


def _bd(block, k_per_b):
    # block: [k_per_b, 16] -> [8*k_per_b, 128] block diagonal over batches
    m = np.zeros((8 * k_per_b, 128), np.float32)
    for b in range(8):
        m[b * k_per_b:(b + 1) * k_per_b, b * 16:(b + 1) * 16] = block
    return m


def _build_weight_bank(conv_w0, conv_w, conv_b, io_w, io_b, mixer_w, mixer_b):
    conv_w0 = conv_w0.astype(np.float32)
    conv_w = conv_w.astype(np.float32)
    conv_b = conv_b.astype(np.float32)
    io_w = io_w.astype(np.float32)
    io_b = io_b.astype(np.float32)
    mixer_w = mixer_w.astype(np.float32)
    mixer_b = mixer_b.astype(np.float32)

    wbank = np.zeros((128, NW), np.float32)
    bbank = np.zeros((128, NBIAS), np.float32)

    # gamma_i: S_i = S~_i + gamma_i
    gam = [0.0] * NL
    for i in range(1, NL):
        gam[i] = 8.0 * gam[i - 1] + io_b[i - 1].sum() / 2.0

    # layer 0 ctrlS strip block: strip r = tap r; strip-local rows 0:8 hold
    # sig/2 (so the sig weight carries a 2x), rows 8:32 hold ctrl.
    o = _O[("cs", 0)]
    for r in range(KSIZE):
        for b in range(8):
            wbank[32 * r + b, o + b * 16:o + (b + 1) * 16] = \
                2.0 * conv_w0[r][0]
            for c in range(NUM_CTRL):
                wbank[32 * r + 8 + b * 3 + c, o + b * 16:o + (b + 1) * 16] = \
                    conv_w0[r][1 + c]
    bbank[:, 0] = np.tile(conv_b[0], 8)

    # u_i blocks (i = 0..15)
    for i in range(16):
        u = io_w[i].sum(axis=1) / 2.0          # [16]
        o = _O[("u", i)]
        for b in range(8):
            wbank[b * 16:(b + 1) * 16, o + b] = u

    # mixer blocks
    for i in range(NL):
        o = _O[("mix", i)]
        for b in range(8):
            wbank[b * 16:(b + 1) * 16, o + b] = mixer_w[i * CH:(i + 1) * CH, 0]
    bbank[:8, NL] = mixer_b[0]

    # layers 1..17: composed taps + ctrlS strip block + bias-hat
    for i in range(1, NL):
        wk = conv_w[i - 1]                     # [K, 19, 16]
        bias = conv_b[i].copy()
        vsum = np.zeros(CH, np.float32)
        ot = _O[("tap", i)]
        for k in range(KSIZE):
            comp = io_w[i - 1] @ wk[k][:CH]    # [16(h), 16(out)]
            wbank[:, ot + k * 128: ot + (k + 1) * 128] = _bd(comp, CH)
            bias += io_b[i - 1] @ wk[k][:CH]
            vsum += wk[k][:CH].sum(axis=0)
        bias += vsum * gam[i - 1]
        bbank[:, i] = np.tile(bias, 8)
        o = _O[("cs", i)]
        for r in range(KSIZE):
            vk = wk[r][:CH].sum(axis=0)        # [16]
            for b in range(8):
                wbank[32 * r + b, o + b * 16:o + (b + 1) * 16] = vk
                for c in range(NUM_CTRL):
                    wbank[32 * r + 8 + b * 3 + c, o + b * 16:o + (b + 1) * 16] = \
                        wk[r][CH + c]

    return wbank.astype(BF16), bbank


def _build_core_cbuf(x_core, layer):
    """x_core: [BL, T, 4] f32 -> [96, W] bf16 ctrl+S buffer for `layer`:
    strip r (rows 32r:32r+32) = rows 0:8 sig/2, rows 8:32 ctrl (8+b*3+c),
    pre-shifted right by (2-r)*d_layer (strip 2 = canonical, shift 0)."""
    x_core = x_core.astype(np.float32)
    base = np.zeros((32, W), np.float32)
    for b in range(BL):
        base[b] = x_core[b, :, 0] * 0.5
        for c in range(NUM_CTRL):
            base[8 + b * 3 + c] = x_core[b, :, 1 + c]
    d = DILATIONS[layer]
    buf = np.zeros((96, W), np.float32)
    for r in range(KSIZE):
        s = (KSIZE - 1 - r) * d
        buf[32 * r:32 * r + 32, s:] = base[:, :W - s]
    return buf.astype(BF16)


def build_graph():
    nc = bacc.Bacc("TRN2", target_bir_lowering=False, debug=False)

    p_c = [nc.declare_dram_parameter(f"cbuf{q}", [96, W], mybir.dt.bfloat16,
                                     isOutput=False) for q in range(3)]
    p_w = nc.declare_dram_parameter("wbank", [128, NW], mybir.dt.bfloat16, isOutput=False)
    p_b = nc.declare_dram_parameter("bbank", [128, NBIAS], mybir.dt.float32, isOutput=False)
    p_out = nc.declare_dram_parameter("out", [8, FRAME], mybir.dt.float32, isOutput=True)

    with tile.TileContext(nc) as tc:
        with (
            tc.tile_pool(name="persist", bufs=1) as persist,
            tc.tile_pool(name="wu", bufs=1) as wu,
            tc.tile_pool(name="ps", bufs=2, space="PSUM") as ps,
            tc.tile_pool(name="sps", bufs=2, space="PSUM") as sps,
            tc.tile_pool(name="mixp", bufs=1, space="PSUM") as mixp,
        ):
            c_sb = [persist.tile([96, W], mybir.dt.bfloat16, tag=f"c{q}",
                                 name=f"c{q}")
                    for q in range(3)]
            w_sb = persist.tile([128, NW], mybir.dt.bfloat16, tag="wbank")
            b_sb = persist.tile([128, NBIAS], mybir.dt.float32, tag="bbank")
            hA = persist.tile([128, W], mybir.dt.bfloat16, tag="hA")
            hB = persist.tile([128, W], mybir.dt.bfloat16, tag="hB")
            out_sb = persist.tile([8, FRAME], mybir.dt.float32, tag="outsb")

            # Front DMAs.  The SDMA engines fair-share bandwidth across ALL
            # outstanding transfers, so only layer 0/1's data goes up front
            # (c0/c1 row-split across the two HWDGE rings); everything else
            # is issued behind a dummy DMA that depends on layer 0's first
            # ReLU, which defers it in queue order until the front loads are
            # done (see the deferred block in the layer-0 body).
            o1 = _O[("tap", 1)]
            nc.sync.dma_start(out=w_sb[:, :144], in_=p_w[:, :144])
            nc.sync.dma_start(out=b_sb[:], in_=p_b[:])
            # Front: layer 0/1's data only.  The SDMA engines fair-share
            # bandwidth across ALL outstanding transfers at packet
            # granularity, so the bulk (weights for layers 2-17, c2) defers
            # behind fake-write gates in the layer-0 body.
            nc.sync.dma_start(out=c_sb[0][0:48], in_=p_c[0][0:48])
            nc.scalar.dma_start(out=c_sb[0][48:96], in_=p_c[0][48:96])
            nc.sync.dma_start(out=w_sb[:, o1:o1 + 528], in_=p_w[:, o1:o1 + 528])
            nc.sync.dma_start(out=c_sb[1][0:48], in_=p_c[1][0:48])
            nc.scalar.dma_start(out=c_sb[1][48:96], in_=p_c[1][48:96])

            mixS = mixp.tile([8, FRAME], mybir.dt.float32, tag="mixS")

            # PE warm-up: throwaway matmuls while the front DMAs stream, so
            # the HAM clock gate is open when real work starts (needs ~3.4us
            # of sustained PE activity).
            warm = wu.tile([128, 128], mybir.dt.bfloat16, tag="warm")
            nc.vector.memset(warm[:], 0.0)
            for _ in range(36):
                wps = ps.tile([128, TT], mybir.dt.float32, tag="hps")
                nc.tensor.matmul(wps[:, :128], warm[:], warm[:],
                                 start=True, stop=True)
                nc.tensor.matmul(wps[:, :128], warm[:], warm[:],
                                 start=True, stop=True)

            pending = []
            gtile = [0]

            def flush(keep=0):
                while len(pending) > keep:
                    for fn in pending.pop(0):
                        fn()

            for i in range(NL):
                d = DILATIONS[i]
                src_c = c_sb[i % 3]
                dst_c = c_sb[(i + 2) % 3] if i <= 15 else None
                src_h = hA if (i - 1) % 2 == 0 else hB
                dst_h = hA if i % 2 == 0 else hB
                lo = LO[i]
                ot = _O.get(("tap", i))
                ocs = _O[("cs", i)]
                # tiles stride from lo (not 512-aligned): avoids the 2-4
                # column leading tiles at trimmed layer heads
                if SIM_ALIGNED_TILES:
                    starts = [max(jj * TT, lo) for jj in range(lo // TT, NTILES)]
                else:
                    starts = list(range(lo, T, TT))
                nt = len(starts)
                for j, a in enumerate(starts):
                    b = min(a + TT, T) if not SIM_ALIGNED_TILES else \
                        min((lo // TT + j + 1) * TT, T)
                    wid = b - a
                    h_ps = ps.tile([128, TT], mybir.dt.float32, tag="hps")
                    if i > 0:
                        for k in range(KSIZE):
                            shift = (KSIZE - 1 - k) * d
                            nc.tensor.matmul(
                                h_ps[:, :wid],
                                w_sb[:, ot + k * 128: ot + (k + 1) * 128],
                                src_h[:, a - shift: a - shift + wid],
                                start=(k == 0),
                                stop=False,
                            )
                    # ctrl+S contribution: one flat K=96 matmul over the
                    # pre-shifted strip buffer.
                    nc.tensor.matmul(
                        h_ps[:, :wid],
                        w_sb[0:96, ocs: ocs + 128],
                        src_c[0:96, a: a + wid],
                        start=(i == 0),
                        stop=True,
                    )
                    if i == 0 and j % 2 == 1:
                        # halve layer-0's serial ReLU chain: odd tiles on DVE
                        nc.vector.tensor_scalar(
                            out=dst_h[:, a: a + wid],
                            in0=h_ps[:, :wid],
                            scalar1=b_sb[:, i:i + 1],
                            scalar2=0.0,
                            op0=mybir.AluOpType.add,
                            op1=mybir.AluOpType.max,
                        )
                    elif b == T:
                        # last tile of a layer: split the ReLU across both
                        # engines to halve its latency - the next layer's
                        # second tile reuses this PSUM buffer and would
                        # otherwise stall on the full-width eviction.
                        h2 = wid // 2
                        nc.scalar.activation(
                            out=dst_h[:, a: a + h2],
                            in_=h_ps[:, :h2],
                            func=mybir.ActivationFunctionType.Relu,
                            bias=b_sb[:, i:i + 1],
                            scale=1.0,
                        )
                        nc.vector.tensor_scalar(
                            out=dst_h[:, a + h2: a + wid],
                            in0=h_ps[:, h2:wid],
                            scalar1=b_sb[:, i:i + 1],
                            scalar2=0.0,
                            op0=mybir.AluOpType.add,
                            op1=mybir.AluOpType.max,
                        )
                    else:
                        nc.scalar.activation(
                            out=dst_h[:, a: a + wid],
                            in_=h_ps[:, :wid],
                            func=mybir.ActivationFunctionType.Relu,
                            bias=b_sb[:, i:i + 1],
                            scale=1.0,
                        )
                    # Deferred bulk loads: the Tile scheduler reorders
                    # dependency-free DMAs to the front where they steal
                    # SDMA bandwidth from c0/c1, so each deferred load is
                    # gated by a tiny "fake write" into its own destination
                    # corner that reads this tile's ReLU output (real WAW
                    # edge; the corners are overwritten by the load itself).
                    if i == 0 and j == 0:
                        for lo_c, hi_c in ((o1 + 528, 4368), (4368, 6768),
                                           (6768, NW)):
                            nc.sync.dma_start(out=w_sb[0:8, lo_c:lo_c + 1],
                                              in_=dst_h[0:8, a:a + 1])
                            nc.sync.dma_start(out=w_sb[:, lo_c:hi_c],
                                              in_=p_w[:, lo_c:hi_c])
                        for r0, r1 in ((8, 32), (40, 64), (72, 96)):
                            nc.scalar.dma_start(out=c_sb[2][r0:r0 + 8, 0:2],
                                                in_=dst_h[0:8, a:a + 2])
                            nc.scalar.dma_start(out=c_sb[2][r0:r1],
                                                in_=p_c[2][r0:r1])
                    # pop epilogues two tiles per visit: adjacent S/mix
                    # matmul pairs share the PE-stream transition cost
                    # phase the two-tile pops so a visit always lands on
                    # each layer's LAST tile: its STTs start during the
                    # final conv instead of inside the congested boundary
                    # drain where vector is head-blocked.
                    if (nt - 1 - j) % 2 == 0:
                        flush(2)

                    def epilogue(i=i, j=j, a=a, b=b, dst_h=dst_h, dst_c=dst_c):
                        # S-update: S~_{i+1}[t] = u_i.h_i[t] (+ 8 S~_i at
                        # evict).  Visits without a mix partner alternate
                        # the PSUM column group so adjacent S matmuls in a
                        # popped pair overlap like S+mix pairs do.
                        has_mix = b > MIX_T0
                        if i <= 15 and b > LO[i + 1]:
                            sa = max(a, LO[i + 1])
                            sw = b - sa
                            so = sa - a
                            o_u = _O[("u", i)]
                            sb0 = 32
                            s_ps = sps.tile([40, TT], mybir.dt.float32, tag="sps")
                            nc.tensor.matmul(
                                s_ps[sb0:sb0 + 8, so: so + sw],
                                w_sb[:, o_u: o_u + 8],
                                dst_h[:, sa: sa + sw],
                                start=True, stop=True,
                                skip_group_check=True,
                            )
                        if has_mix:
                            ma = max(a, MIX_T0)
                            o_m = _O[("mix", i)]
                            w0, w1 = ma - MIX_T0, b - MIX_T0
                            if SIM_SPLIT_MIX:
                                cuts = [w0] + [c for c in (512, 1024, 1536)
                                               if w0 < c < w1] + [w1]
                            else:
                                cuts = [w0, w1]
                            for c0_, c1_ in zip(cuts, cuts[1:]):
                                nc.tensor.matmul(
                                    mixS[0:8, c0_:c1_],
                                    w_sb[:, o_m: o_m + 8],
                                    dst_h[:, MIX_T0 + c0_: MIX_T0 + c1_],
                                    start=(i == 0),
                                    stop=(i == NL - 1),
                                    skip_group_check=True,
                                )
                            # stream the final eviction behind layer 17's
                            # last mix matmuls (chunk complete => evict)
                            if i == NL - 1:
                                c0, c1 = max(0, a - MIX_T0), b - MIX_T0
                                if j in (1, 3):
                                    nc.vector.tensor_scalar_add(
                                        out=out_sb[:, c0:c1],
                                        in0=mixS[0:8, c0:c1],
                                        scalar1=b_sb[:8, NL: NL + 1],
                                    )
                                else:
                                    nc.scalar.activation(
                                        out=out_sb[:, c0:c1],
                                        in_=mixS[0:8, c0:c1],
                                        func=mybir.ActivationFunctionType.Identity,
                                        bias=b_sb[:8, NL: NL + 1],
                                        scale=1.0,
                                    )
                                nc.sync.dma_start(out=p_out[:, c0:c1],
                                                  in_=out_sb[:, c0:c1])
                        if i <= 15 and b > LO[i + 1]:
                            # S recursion into layer (i+2)'s buffer, strip 2
                            # (strip-local rows 0:8 = partitions 64:72).
                            nc.vector.scalar_tensor_tensor(
                                out=dst_c[64:72, sa: sa + sw],
                                in0=c_sb[(i + 1) % 3][64:72, sa: sa + sw],
                                scalar=8.0,
                                in1=s_ps[sb0:sb0 + 8, so: so + sw],
                                op0=mybir.AluOpType.mult,
                                op1=mybir.AluOpType.add,
                            )

                    pending.append([epilogue])

                if i <= 15:
                    # shift-replicate S~_{i+1} from canonical strip 2 into
                    # strips 0/1 of layer (i+2)'s buffer (gpsimd DGE queue),
                    # and re-shift that buffer's static ctrl rows for layer
                    # (i+2)'s dilation (sync queue, idle after startup).
                    def s_repl(i=i, dst_c=dst_c):
                        lo1 = LO[i + 1]
                        d2 = DILATIONS[i + 2]
                        for r in range(2):
                            s = (KSIZE - 1 - r) * d2
                            nc.gpsimd.dma_start(
                                out=dst_c[32 * r: 32 * r + 8, lo1 + s: T],
                                in_=dst_c[64:72, lo1: T - s],
                            )

                    pending[-1].append(s_repl)
                    if i >= 1:
                        def ctrl_prep(i=i, dst_c=dst_c):
                            lo2 = LO[i + 2]
                            d2 = DILATIONS[i + 2]
                            for r in range(2):
                                s = (KSIZE - 1 - r) * d2
                                nc.sync.dma_start(
                                    out=dst_c[32 * r + 8: 32 * r + 32, lo2: T],
                                    in_=dst_c[72:96, lo2 - s: T - s],
                                )

                        pending[-1].append(ctrl_prep)

            flush()
            # out DMAs streamed inside the layer-17 epilogues

    nc.finalize()
    return nc


_CACHE = {}


def kernel(**inputs) -> np.ndarray:
    inp = inputs["input"].astype(np.float32)          # [64, 4092, 4]
    wbank, bbank = _build_weight_bank(
        inputs["conv_w0"], inputs["conv_w"], inputs["conv_b"],
        inputs["io_w"], inputs["io_b"], inputs["mixer_w"], inputs["mixer_b"],
    )

    if "nc" not in _CACHE:
        _CACHE["nc"] = build_graph()
    nc = _CACHE["nc"]

    in_maps = []
    for c in range(NCORES):
        xc = inp[c * BL:(c + 1) * BL]
        m = {"wbank": wbank, "bbank": bbank}
        for q in range(3):
            m[f"cbuf{q}"] = _build_core_cbuf(xc, q)
        in_maps.append(m)

    res = run_bass_kernel_spmd(nc, in_maps, core_ids=list(range(NCORES)))
    outs = [res.results[c]["out"] for c in range(NCORES)]       # each [8, 2048]
    full = np.concatenate(outs, axis=0)                         # [64, 2048]
    return full[:, :, None].astype(np.float32)                  # [64, 2048, 1]


if __name__ == "__main__":
    data = np.load("/root/problem/ref_inputs.npz")
    out = kernel(**{k: data[k] for k in data.files})
    ref = np.load("/root/problem/ref_out.npy")
    err = np.linalg.norm(out - ref) / np.linalg.norm(ref)
    print("Relative error:", err)
